# revision 5
# baseline (speedup 1.0000x reference)
"""Trainium2 Bass kernel for nn_BaseLineModel (hierarchical sentence->doc model).

Pipeline per core (4 docs, 128 sentences):
  embedding gather (indirect DMA, fp32) -> PE transpose to [E, tok] -> bf16
  -> conv as 5 shifted matmuls over 4-sentence groups -> tanh
  -> attn over tokens (softmax via ACT exp w/ fused accum) -> s0
  -> x-projection matmul -> LSTM over sentence positions (batch=4 docs)
  -> masked attn over sentences -> sigmoid output.

Data-parallel over docs: core k handles docs 4k..4k+3 end-to-end; host
concatenates the 8 [4,1] outputs. No collectives.
"""
import sys

for _p in ("/opt/trn_rl_repo", "/root/.axon_site/_ro/trn_rl_repo"):
    if _p not in sys.path:
        sys.path.insert(0, _p)

from contextlib import ExitStack

import numpy as np
import ml_dtypes

import concourse.bass as bass
import concourse.tile as tile
from concourse import mybir
from concourse.bass import IndirectOffsetOnAxis
from concourse.bass_utils import run_bass_kernel_spmd
from concourse.masks import make_identity
from concourse.tile import TileContext

from concourse.vector_clock import ScopedClock


class _TC(TileContext):
    """TileContext that limits every instruction to a single sem wait
    (this walrus rejects multiple sync waits on one instruction); extra
    waits are spilled onto preceding same-engine nops."""

    def _commit_instruction(self, inst, lazy_reg_writes: bool = True):
        si = getattr(inst, "sync_info", None)
        if (
            si is not None
            and si.on_wait
            and len(si.on_wait) > 1
            and inst.engine != mybir.EngineType.Unassigned
        ):
            waits = list(si.on_wait)
            inst.sync_info = mybir.SyncInfo(
                on_wait=[waits[-1]], on_update=list(si.on_update or []))
            eng = self.nc.engines[inst.engine]
            for w in waits[:-1]:
                nop = eng.nop().ins
                nop.sync_info = mybir.SyncInfo(on_wait=[w], on_update=[])
        return super()._commit_instruction(inst, lazy_reg_writes)

    def _drain_and_barrier(self, tick_clock, wait_clock):
        carrier = self.nc.sync.nop().ins
        wait_clock.add_sem_waits(
            carrier, ScopedClock({None: tick_clock.global_clock}))
        si = carrier.sync_info
        if si is not None and si.on_wait and len(si.on_wait) > 1:
            waits = list(si.on_wait)
            carrier.sync_info = mybir.SyncInfo(
                on_wait=[waits[0]], on_update=list(si.on_update or []))
            for w in waits[1:]:
                n2 = self.nc.sync.nop().ins
                n2.sync_info = mybir.SyncInfo(on_wait=[w], on_update=[])
        self.nc.sync.drain()
        self.nc.all_engine_barrier()
        assert self.sems is not None
        popped = self.nc._tile_sem_poison_stack.pop()
        assert popped is self._sem_poison
        self.nc.clear_and_free_semaphores(list(self.sems.allocated().values()))
        self.nc.all_engine_barrier()


BF16 = mybir.dt.bfloat16
FP32 = mybir.dt.float32
INT32 = mybir.dt.int32
AF = mybir.ActivationFunctionType
OP = mybir.AluOpType

B, S, L = 32, 64, 128
TOTAL = B * L // L * 32  # 1024 (32 sentences/doc nominal)
V, E, F, W, H = 30000, 300, 256, 5, 256
T = L - W + 1            # 124 valid conv positions
NCORES = 8
DPC = B // NCORES        # 4 docs per core
ECH = [128, 128, 44]     # E=300 split into K-chunks
bf16 = ml_dtypes.bfloat16


def build_nc(S_eff: int, n_groups: int, group_targets, debug_taps=False):
    """group_targets[g] = list of 4 entries: (t, d) slot or None (pad),
    shared by all cores."""
    NLOC = n_groups * 4
    S4 = S_eff * DPC

    nc = bass.Bass()
    dbg = {}
    if debug_taps:
        dbg["emb"] = nc.dram_tensor("dbg_emb", [128, 4, E], FP32,
                                    kind="ExternalOutput")
        dbg["ebt"] = nc.dram_tensor("dbg_ebt", [128, 3, 512], BF16,
                                    kind="ExternalOutput")
        dbg["c0"] = nc.dram_tensor("dbg_c0", [128, 2, 512], BF16,
                                   kind="ExternalOutput")
        dbg["s0"] = nc.dram_tensor("dbg_s0", [128, 2, S_eff, DPC], BF16,
                                   kind="ExternalOutput")
        dbg["xpt"] = nc.dram_tensor("dbg_xpt", [128, 8, S4], FP32,
                                    kind="ExternalOutput")
        dbg["hs"] = nc.dram_tensor("dbg_hs", [128, S_eff, 8], FP32,
                                   kind="ExternalOutput")
        dbg["l1"] = nc.dram_tensor("dbg_l1", [128, 2, S4], FP32,
                                   kind="ExternalOutput")
        dbg["ex1"] = nc.dram_tensor("dbg_ex1", [128, 2, S_eff, DPC], FP32,
                                    kind="ExternalOutput")
        dbg["den1"] = nc.dram_tensor("dbg_den1", [128, 2, DPC], FP32,
                                     kind="ExternalOutput")
        dbg["num1"] = nc.dram_tensor("dbg_num1", [128, 2, DPC], FP32,
                                     kind="ExternalOutput")
        dbg["s1"] = nc.dram_tensor("dbg_s1", [128, 2, DPC], BF16,
                                   kind="ExternalOutput")
    # ---- DRAM I/O ----
    d_idx = nc.dram_tensor("idx_t", [128, NLOC], INT32, kind="ExternalInput")
    d_wemb = nc.dram_tensor("wemb", [V, E], FP32, kind="ExternalInput")
    d_wconv = nc.dram_tensor("wconvT", [128, W, 3, 2, 128], BF16, kind="ExternalInput")
    d_bconv = nc.dram_tensor("bconv_t", [128, 2], FP32, kind="ExternalInput")
    d_wa0 = nc.dram_tensor("wa0_t", [128, 2, 2, 128], BF16, kind="ExternalInput")
    d_ba0 = nc.dram_tensor("ba0_t", [128, 2], FP32, kind="ExternalInput")
    d_wih = nc.dram_tensor("wih_t", [128, 2, 8, 128], BF16, kind="ExternalInput")
    d_bihhh = nc.dram_tensor("bihhh_t", [128, 8], FP32, kind="ExternalInput")
    d_whh = nc.dram_tensor("whh_t", [128, 2, 8, 128], BF16, kind="ExternalInput")
    d_wa1 = nc.dram_tensor("wa1_t", [128, 2, 2, 128], BF16, kind="ExternalInput")
    d_ba1 = nc.dram_tensor("ba1_t", [128, 2], FP32, kind="ExternalInput")
    d_mb1 = nc.dram_tensor("mb1", [1, S4], FP32, kind="ExternalInput")
    d_wo = nc.dram_tensor("wo_t", [128, 2], BF16, kind="ExternalInput")
    d_boh = nc.dram_tensor("bo_half", [1, 1], FP32, kind="ExternalInput")
    d_out = nc.dram_tensor("out", [1, DPC], FP32, kind="ExternalOutput")

    with _TC(nc) as tc, ExitStack() as ctx:
        consts = ctx.enter_context(tc.tile_pool(name="consts", bufs=1))

        ident = consts.tile([128, 128], FP32)
        make_identity(nc, ident[:, :])
        idx_sb = consts.tile([128, NLOC], INT32)
        nc.sync.dma_start(out=idx_sb[:, :], in_=d_idx[:, :])
        wconv_sb = consts.tile([128, W, 3, 2, 128], BF16)
        nc.sync.dma_start(out=wconv_sb[:, :, :, :, :], in_=d_wconv[:, :, :, :, :])
        bconv_sb = consts.tile([128, 2], FP32)
        nc.sync.dma_start(out=bconv_sb[:, :], in_=d_bconv[:, :])
        wa0_sb = consts.tile([128, 2, 2, 128], BF16)
        nc.sync.dma_start(out=wa0_sb[:, :, :, :], in_=d_wa0[:, :, :, :])
        ba0_sb = consts.tile([128, 2], FP32)
        nc.sync.dma_start(out=ba0_sb[:, :], in_=d_ba0[:, :])
        wih_sb = consts.tile([128, 2, 8, 128], BF16)
        nc.sync.dma_start(out=wih_sb[:, :, :, :], in_=d_wih[:, :, :, :])
        bihhh_sb = consts.tile([128, 8], FP32)
        nc.sync.dma_start(out=bihhh_sb[:, :], in_=d_bihhh[:, :])
        whh_sb = consts.tile([128, 2, 8, 128], BF16)
        nc.sync.dma_start(out=whh_sb[:, :, :, :], in_=d_whh[:, :, :, :])
        wa1_sb = consts.tile([128, 2, 2, 128], BF16)
        nc.sync.dma_start(out=wa1_sb[:, :, :, :], in_=d_wa1[:, :, :, :])
        ba1_sb = consts.tile([128, 2], FP32)
        nc.sync.dma_start(out=ba1_sb[:, :], in_=d_ba1[:, :])
        mb1_sb = consts.tile([128, S4], FP32)
        nc.sync.dma_start(
            out=mb1_sb[:, :], in_=d_mb1[:, :].to_broadcast([128, S4]))
        wo_sb = consts.tile([128, 2], BF16)
        nc.sync.dma_start(out=wo_sb[:, :], in_=d_wo[:, :])
        boh_sb = consts.tile([1, 1], FP32)
        nc.sync.dma_start(out=boh_sb[:, :], in_=d_boh[:, :])

        # persistent accumulators
        s0T_sb = consts.tile([128, 2, S_eff, DPC], BF16)
        nc.vector.memset(s0T_sb[:, :, :, :], 0.0)
        s0scr = consts.tile([128, 2, 4], BF16)

        # ================= Phase A: conv + token attention =================
        with (
            nc.named_scope("phaseA"),
            tc.tile_pool(name="emb", bufs=3) as p_emb,
            tc.tile_pool(name="ebts", bufs=2) as p_ebts,
            tc.tile_pool(name="acts", bufs=2) as p_acts,
            tc.tile_pool(name="small", bufs=2) as p_small,
            tc.tile_pool(name="ebtp", bufs=1, space="PSUM") as p_ebtp,
            tc.tile_pool(name="mmp", bufs=5, space="PSUM") as p_mmp,
        ):
            for g in range(n_groups):
                # ---- gather 4 sentences of embeddings: [tok, sent, E] ----
                emb_g = p_emb.tile([128, 4, E], FP32, tag="emb")
                for s in range(4):
                    nc.gpsimd.indirect_dma_start(
                        out=emb_g[:, s, :],
                        out_offset=None,
                        in_=d_wemb[:, :],
                        in_offset=IndirectOffsetOnAxis(
                            ap=idx_sb[:, 4 * g + s:4 * g + s + 1], axis=0),
                    )
                if debug_taps and g == 0:
                    nc.sync.dma_start(out=dbg["emb"][:, :, :],
                                      in_=emb_g[:, :, :])
                # ---- transpose to ebT [E-chunk, 4*128 tok] (PE) + cast bf16 ----
                ebt_sb = p_ebts.tile([128, 3, 512], BF16, tag="ebts")
                for ec in range(3):
                    ecw = ECH[ec]
                    ebt_ps = p_ebtp.tile([128, 512], FP32, tag=f"ebtp{ec}")
                    for s in range(4):
                        nc.tensor.matmul(
                            out=ebt_ps[:ecw, 128 * s:128 * (s + 1)],
                            lhsT=emb_g[:, s, 128 * ec:128 * ec + ecw],
                            rhs=ident[:, :],
                            start=True, stop=True,
                        )
                    nc.vector.tensor_copy(
                        out=ebt_sb[:ecw, ec, :], in_=ebt_ps[:ecw, :])
                if debug_taps and g == 0:
                    nc.sync.dma_start(out=dbg["ebt"][:, :, :],
                                      in_=ebt_sb[:, :, :])
                # ---- conv: accumulate 5 shifted matmuls x 3 K-chunks ----
                c0_sb = p_acts.tile([128, 2, 512], BF16, tag="c0")
                for fc in range(2):
                    c0_ps = p_mmp.tile([128, 512], FP32, tag="mm")
                    nmm = W * 3
                    ki = 0
                    for w in range(W):
                        for ec in range(3):
                            ecw = ECH[ec]
                            nc.tensor.matmul(
                                out=c0_ps[:, 0:508],
                                lhsT=wconv_sb[:ecw, w, ec, fc, :],
                                rhs=ebt_sb[:ecw, ec, w:w + 508],
                                start=(ki == 0), stop=(ki == nmm - 1),
                            )
                            ki += 1
                    nc.scalar.activation(
                        out=c0_sb[:, fc, 0:508], in_=c0_ps[:, 0:508],
                        func=AF.Tanh, bias=bconv_sb[:, fc:fc + 1])
                if debug_taps and g == 0:
                    nc.sync.dma_start(out=dbg["c0"][:, :, :],
                                      in_=c0_sb[:, :, :])
                # ---- attn0 logits: Wa0.T @ c0T, tanh ----
                lg_sb = p_acts.tile([128, 2, 512], BF16, tag="lg")
                for mc in range(2):
                    lg_ps = p_mmp.tile([128, 512], FP32, tag="mm")
                    for kc in range(2):
                        nc.tensor.matmul(
                            out=lg_ps[:, 0:508],
                            lhsT=wa0_sb[:, kc, mc, :],
                            rhs=c0_sb[:, kc, 0:508],
                            start=(kc == 0), stop=(kc == 1),
                        )
                    nc.scalar.activation(
                        out=lg_sb[:, mc, 0:508], in_=lg_ps[:, 0:508],
                        func=AF.Tanh, bias=ba0_sb[:, mc:mc + 1])
                # ---- softmax over tokens + weighted sum (per sentence) ----
                ex_sb = p_acts.tile([128, 2, 512], BF16, tag="ex")
                den_sb = p_small.tile([128, 2, 4], FP32, tag="den")
                num_sb = p_small.tile([128, 2, 4], FP32, tag="num")
                scr_sb = p_small.tile([128, 128], BF16, tag="scr")
                for mc in range(2):
                    for s in range(4):
                        nc.scalar.activation(
                            out=ex_sb[:, mc, 128 * s:128 * s + T],
                            in_=lg_sb[:, mc, 128 * s:128 * s + T],
                            func=AF.Exp,
                            accum_out=den_sb[:, mc, s:s + 1])
                nc.vector.reciprocal(
                    out=den_sb[:, :, :], in_=den_sb[:, :, :])
                for mc in range(2):
                    for s in range(4):
                        nc.vector.scalar_tensor_tensor(
                            out=scr_sb[:, 0:T],
                            in0=ex_sb[:, mc, 128 * s:128 * s + T],
                            scalar=1.0,
                            in1=c0_sb[:, mc, 128 * s:128 * s + T],
                            op0=OP.mult, op1=OP.mult,
                            accum_out=num_sb[:, mc, s:s + 1])
                # ---- s0 = num/den scattered to (t, d) slots ----
                tgt = group_targets[g]
                runs = _target_runs(tgt)
                for mc in range(2):
                    for (s_lo, n_run, td) in runs:
                        if td is None:
                            out_ap = s0scr[:, mc, s_lo:s_lo + n_run]
                        else:
                            t0, d0 = td
                            out_ap = s0T_sb[:, mc, t0:t0 + n_run, d0:d0 + 1]
                        nc.vector.tensor_tensor(
                            out=out_ap,
                            in0=num_sb[:, mc, s_lo:s_lo + n_run],
                            in1=den_sb[:, mc, s_lo:s_lo + n_run],
                            op=OP.mult)

        if debug_taps:
            nc.sync.dma_start(out=dbg["s0"][:, :, :, :],
                              in_=s0T_sb[:, :, :, :])

        # ================= Phase B: x-projection =================
        xpt_sb = consts.tile([128, 8, S4], FP32)
        with nc.named_scope("phaseB"), \
                tc.tile_pool(name="xpp", bufs=4, space="PSUM") as p_xpp:
            for gt in range(8):
                xp_ps = p_xpp.tile([128, S4], FP32, tag="xp")
                for kc in range(2):
                    nc.tensor.matmul(
                        out=xp_ps[:, :],
                        lhsT=wih_sb[:, kc, gt, :],
                        rhs=s0T_sb[:, kc, :, :],
                        start=(kc == 0), stop=(kc == 1),
                    )
                nc.scalar.activation(
                    out=xpt_sb[:, gt, :], in_=xp_ps[:, :],
                    func=AF.Identity, bias=bihhh_sb[:, gt:gt + 1])

        # ================= Phase C: LSTM over sentence slots =================
        hT_sb = consts.tile([128, 2, DPC], BF16)
        nc.vector.memset(hT_sb[:, :, :], 0.0)
        c_sb = consts.tile([128, 8], FP32)
        nc.vector.memset(c_sb[:, :], 0.0)
        hs_sb = consts.tile([128, S_eff, 8], FP32)
        hsb_sb = consts.tile([128, S_eff, 2, DPC], BF16)
        with (
            nc.named_scope("phaseC"),
            tc.tile_pool(name="gp", bufs=2, space="PSUM") as p_gp,
            tc.tile_pool(name="lst", bufs=2) as p_lst,
        ):
            for t in range(S_eff):
                g_ps = p_gp.tile([128, 32], FP32, tag="g")
                for gt in range(8):
                    for kc in range(2):
                        nc.tensor.matmul(
                            out=g_ps[:, 4 * gt:4 * gt + 4],
                            lhsT=whh_sb[:, kc, gt, :],
                            rhs=hT_sb[:, kc, :],
                            start=(kc == 0), stop=(kc == 1),
                        )
                ga = p_lst.tile([128, 32], FP32, tag="ga")
                nc.vector.tensor_tensor(
                    out=ga[:, :], in0=g_ps[:, :],
                    in1=xpt_sb[:, :, DPC * t:DPC * (t + 1)], op=OP.add)
                nc.scalar.activation(out=ga[:, 0:16], in_=ga[:, 0:16],
                                     func=AF.Sigmoid)
                nc.scalar.activation(out=ga[:, 16:24], in_=ga[:, 16:24],
                                     func=AF.Tanh)
                nc.scalar.activation(out=ga[:, 24:32], in_=ga[:, 24:32],
                                     func=AF.Sigmoid)
                tmp = p_lst.tile([128, 8], FP32, tag="tmp")
                nc.vector.tensor_tensor(
                    out=tmp[:, :], in0=ga[:, 0:8], in1=ga[:, 16:24], op=OP.mult)
                nc.vector.tensor_tensor(
                    out=c_sb[:, :], in0=c_sb[:, :], in1=ga[:, 8:16], op=OP.mult)
                nc.vector.tensor_tensor(
                    out=c_sb[:, :], in0=c_sb[:, :], in1=tmp[:, :], op=OP.add)
                tch = p_lst.tile([128, 8], FP32, tag="tch")
                nc.scalar.activation(out=tch[:, :], in_=c_sb[:, :], func=AF.Tanh)
                nc.vector.tensor_tensor(
                    out=hs_sb[:, t, :], in0=ga[:, 24:32], in1=tch[:, :],
                    op=OP.mult)
                nc.vector.tensor_copy(
                    out=hsb_sb[:, t, :, :],
                    in_=hs_sb[:, t, :].rearrange("p (h d) -> p h d", h=2))
                nc.vector.tensor_copy(out=hT_sb[:, :, :], in_=hsb_sb[:, t, :, :])

        if debug_taps:
            nc.sync.dma_start(out=dbg["xpt"][:, :, :], in_=xpt_sb[:, :, :])
            nc.sync.dma_start(out=dbg["hs"][:, :, :], in_=hs_sb[:, :, :])

        # ================= Phase D: sentence attention + output ============
        with (
            nc.named_scope("phaseD"),
            tc.tile_pool(name="a1p", bufs=2, space="PSUM") as p_a1p,
            tc.tile_pool(name="a1s", bufs=2) as p_a1s,
        ):
            l1_sb = p_a1s.tile([128, 2, S4], FP32, tag="l1")
            for mc in range(2):
                l1_ps = p_a1p.tile([128, S4], FP32, tag="l1p")
                for kc in range(2):
                    nc.tensor.matmul(
                        out=l1_ps[:, :],
                        lhsT=wa1_sb[:, kc, mc, :],
                        rhs=hsb_sb[:, :, kc, :],
                        start=(kc == 0), stop=(kc == 1),
                    )
                nc.scalar.activation(
                    out=l1_sb[:, mc, :], in_=l1_ps[:, :],
                    func=AF.Tanh, bias=ba1_sb[:, mc:mc + 1])
                nc.vector.tensor_tensor(
                    out=l1_sb[:, mc, :], in0=l1_sb[:, mc, :],
                    in1=mb1_sb[:, :], op=OP.add)
            ex1 = p_a1s.tile([128, 2, S_eff, DPC], FP32, tag="ex1")
            den1 = p_a1s.tile([128, 2, DPC], FP32, tag="den1")
            num1 = p_a1s.tile([128, 2, DPC], FP32, tag="num1")
            scr1 = p_a1s.tile([128, S_eff], FP32, tag="scr1")
            l1v = l1_sb.rearrange("p m (t d) -> p m t d", d=DPC)
            for mc in range(2):
                for d in range(DPC):
                    nc.scalar.activation(
                        out=ex1[:, mc, :, d], in_=l1v[:, mc, :, d],
                        func=AF.Exp, accum_out=den1[:, mc, d:d + 1])
            nc.vector.reciprocal(out=den1[:, :, :], in_=den1[:, :, :])
            for mc in range(2):
                for d in range(DPC):
                    nc.vector.scalar_tensor_tensor(
                        out=scr1[:, :],
                        in0=ex1[:, mc, :, d],
                        scalar=1.0,
                        in1=hsb_sb[:, :, mc, d],
                        op0=OP.mult, op1=OP.mult,
                        accum_out=num1[:, mc, d:d + 1])
            s1_sb = p_a1s.tile([128, 2, DPC], BF16, tag="s1")
            nc.vector.tensor_tensor(
                out=s1_sb[:, :, :], in0=num1[:, :, :], in1=den1[:, :, :],
                op=OP.mult)
            if debug_taps:
                nc.sync.dma_start(out=dbg["l1"][:, :, :], in_=l1_sb[:, :, :])
                nc.sync.dma_start(out=dbg["ex1"][:, :, :, :], in_=ex1[:, :, :, :])
                nc.sync.dma_start(out=dbg["den1"][:, :, :], in_=den1[:, :, :])
                nc.sync.dma_start(out=dbg["num1"][:, :, :], in_=num1[:, :, :])
                nc.sync.dma_start(out=dbg["s1"][:, :, :], in_=s1_sb[:, :, :])
            o_ps = p_a1p.tile([128, DPC], FP32, tag="op")
            for kc in range(2):
                nc.tensor.matmul(
                    out=o_ps[:1, :],
                    lhsT=wo_sb[:, kc:kc + 1],
                    rhs=s1_sb[:, kc, :],
                    start=(kc == 0), stop=(kc == 1),
                )
            y_sb = p_a1s.tile([1, DPC], FP32, tag="y")
            nc.scalar.activation(
                out=y_sb[:, :], in_=o_ps[:1, :],
                func=AF.Tanh, bias=boh_sb[:1, :1], scale=0.5)
            nc.vector.tensor_scalar(
                out=y_sb[:, :], in0=y_sb[:, :],
                scalar1=0.5, scalar2=0.5, op0=OP.mult, op1=OP.add)
            nc.sync.dma_start(out=d_out[:, :], in_=y_sb[:, :])

    return nc


def _target_runs(tgt):
    """Compress 4 per-sentence (t, d)/None targets into (start, len, td) runs
    where a run covers consecutive t at fixed d (or None-pads)."""
    runs = []
    i = 0
    while i < 4:
        if tgt[i] is None:
            j = i
            while j < 4 and tgt[j] is None:
                j += 1
            runs.append((i, j - i, None))
            i = j
        else:
            t0, d0 = tgt[i]
            j = i + 1
            while j < 4 and tgt[j] is not None and tgt[j] == (t0 + (j - i), d0):
                j += 1
            runs.append((i, j - i, (t0, d0)))
            i = j
    return runs


def _host_prep(inputs):
    inp = {k: np.asarray(v) for k, v in inputs.items()}
    tok = inp["input"].astype(np.int32)
    num_sent = inp["num_sent"].astype(np.int64)
    mask = np.asarray(inp["mask"], np.float32)

    S_eff = max(int(num_sent.max()), 1)
    # ragged mapping exactly like the reference scatter
    batch_ids = np.repeat(np.arange(B), num_sent)
    if len(batch_ids) < TOTAL:
        batch_ids = np.concatenate(
            [batch_ids, np.full(TOTAL - len(batch_ids), B - 1, np.int64)])
    batch_ids = batch_ids[:TOTAL]
    offsets = np.cumsum(num_sent) - num_sent
    pos = np.arange(TOTAL) - offsets[batch_ids]
    valid = pos < num_sent[batch_ids]

    per_core = []
    for k in range(NCORES):
        sids = np.where((batch_ids // DPC == k) & valid)[0]
        per_core.append([(int(j), int(batch_ids[j] % DPC), int(pos[j]))
                         for j in sids])
    n_groups = max(1, (max(len(pc) for pc in per_core) + 3) // 4)
    NLOC = n_groups * 4

    # per-core group target maps; must agree across cores for the shared
    # program (true for uniform num_sent). Fall back handled by caller.
    tmaps = []
    for k in range(NCORES):
        tm = []
        for j in range(NLOC):
            if j < len(per_core[k]):
                _, d, p = per_core[k][j]
                tm.append((p, d))
            else:
                tm.append(None)
        tmaps.append(tm)
    uniform = all(tm == tmaps[0] for tm in tmaps)

    group_targets = [tmaps[0][4 * g:4 * g + 4] for g in range(n_groups)]

    in_maps = []
    wemb = np.ascontiguousarray(inp["Wemb"], np.float32)
    wconvT = np.zeros((128, W, 3, 2, 128), bf16)
    wc = np.asarray(inp["Wconv"], np.float32)  # [F,1,W,E]
    for ec in range(3):
        ecw = ECH[ec]
        for fc in range(2):
            # [p, w, m] = Wconv[fc*128+m, 0, w, ec*128+p]
            blk = wc[128 * fc:128 * (fc + 1), 0, :, 128 * ec:128 * ec + ecw]
            wconvT[:ecw, :, ec, fc, :] = blk.transpose(2, 1, 0).astype(bf16)
    bconv_t = np.asarray(inp["bconv"], np.float32).reshape(2, 128).T.copy()
    wa0_t = _pack_kx(inp["Wa0"])
    ba0_t = np.asarray(inp["ba0"], np.float32).reshape(2, 128).T.copy()
    wih_t = _pack_kx(np.asarray(inp["Wih"], np.float32).T)   # [F, 4H]
    whh_t = _pack_kx(np.asarray(inp["Whh"], np.float32).T)   # [H, 4H]
    bihhh_t = (np.asarray(inp["bih"], np.float32)
               + np.asarray(inp["bhh"], np.float32)).reshape(8, 128).T.copy()
    wa1_t = _pack_kx(inp["Wa1"])
    ba1_t = np.asarray(inp["ba1"], np.float32).reshape(2, 128).T.copy()
    wo_t = np.asarray(inp["Wo"], np.float32).reshape(2, 128).T.astype(bf16).copy()
    bo_half = (0.5 * np.asarray(inp["bo"], np.float32)).reshape(1, 1)

    for k in range(NCORES):
        idx_t = np.zeros((128, NLOC), np.int32)
        for j, (sj, _, _) in enumerate(per_core[k]):
            idx_t[:, j] = tok[sj]
        mb1 = np.zeros((1, S_eff * DPC), np.float32)
        for d in range(DPC):
            doc = k * DPC + d
            mvals = mask[doc, :S_eff, 0]
            mb1[0, np.arange(S_eff) * DPC + d] = np.where(mvals > 0, 0.0, -1e9)
        in_maps.append({
            "idx_t": idx_t, "wemb": wemb, "wconvT": wconvT,
            "bconv_t": bconv_t, "wa0_t": wa0_t, "ba0_t": ba0_t,
            "wih_t": wih_t, "bihhh_t": bihhh_t, "whh_t": whh_t,
            "wa1_t": wa1_t, "ba1_t": ba1_t, "mb1": mb1,
            "wo_t": wo_t, "bo_half": bo_half,
        })
    return S_eff, n_groups, group_targets, tmaps, uniform, in_maps


def _pack_kx(w):
    """[K=256, M_total] -> [128, kc, mt, 128] tile pack (bf16)."""
    w = np.asarray(w, np.float32)
    K, M = w.shape
    assert K == 256 and M % 128 == 0
    mt = M // 128
    out = np.zeros((128, 2, mt, 128), bf16)
    for kc in range(2):
        for m in range(mt):
            out[:, kc, m, :] = w[128 * kc:128 * (kc + 1),
                                 128 * m:128 * (m + 1)].astype(bf16)
    return out


_NC_CACHE = {}


def kernel(**inputs) -> np.ndarray:
    S_eff, n_groups, group_targets, tmaps, uniform, in_maps = _host_prep(inputs)

    out = np.zeros((B, 1), np.float32)
    if uniform:
        key = (S_eff, n_groups, tuple(tuple(t) if t else None
                                      for g in group_targets for t in g))
        if key not in _NC_CACHE:
            _NC_CACHE[key] = build_nc(S_eff, n_groups, group_targets)
        nc = _NC_CACHE[key]
        res = run_bass_kernel_spmd(nc, in_maps, core_ids=list(range(NCORES)))
        for k in range(NCORES):
            out[k * DPC:(k + 1) * DPC, 0] = res.results[k]["out"][0]
    else:
        # ragged fallback: per-core programs
        from concourse.bass_utils import run_bass_kernel
        for k in range(NCORES):
            gt_k = [tmaps[k][4 * g:4 * g + 4] for g in range(n_groups)]
            nc = build_nc(S_eff, n_groups, gt_k)
            r = run_bass_kernel(nc, in_maps[k], core_id=0)
            out[k * DPC:(k + 1) * DPC, 0] = r["out"][0]
    return out



# revision 6
# speedup vs baseline: 1.4171x; 1.4171x over previous
"""Trainium2 Bass kernel for nn_BaseLineModel (hierarchical sentence->doc model).

v2: bf16 embedding gather (half DMA, bf16 PE transposes, FWL everywhere),
batched softmax (segmented tensor_reduce instead of per-sentence accum),
and the x-projection + LSTM interleaved into the conv group loop
(position-major groups) so their latency hides under conv PE work.

Data-parallel over docs: core k handles docs 4k..4k+3 end-to-end; host
concatenates the 8 [4,1] outputs. No collectives.
"""
import sys

for _p in ("/opt/trn_rl_repo", "/root/.axon_site/_ro/trn_rl_repo"):
    if _p not in sys.path:
        sys.path.insert(0, _p)

from contextlib import ExitStack

import numpy as np
import ml_dtypes

import concourse.bass as bass
import concourse.tile as tile
from concourse import mybir
from concourse.bass import IndirectOffsetOnAxis
from concourse.bass_utils import run_bass_kernel_spmd
from concourse.masks import make_identity
from concourse.tile import TileContext

from concourse.vector_clock import ScopedClock


class _TC(TileContext):
    """TileContext that limits every instruction to a single sem wait
    (this walrus rejects multiple sync waits on one instruction); extra
    waits are spilled onto preceding same-engine nops."""

    def _commit_instruction(self, inst, lazy_reg_writes: bool = True):
        si = getattr(inst, "sync_info", None)
        if (
            si is not None
            and si.on_wait
            and len(si.on_wait) > 1
            and inst.engine != mybir.EngineType.Unassigned
        ):
            waits = list(si.on_wait)
            inst.sync_info = mybir.SyncInfo(
                on_wait=[waits[-1]], on_update=list(si.on_update or []))
            eng = self.nc.engines[inst.engine]
            for w in waits[:-1]:
                nop = eng.nop().ins
                nop.sync_info = mybir.SyncInfo(on_wait=[w], on_update=[])
        return super()._commit_instruction(inst, lazy_reg_writes)

    def _drain_and_barrier(self, tick_clock, wait_clock):
        carrier = self.nc.sync.nop().ins
        wait_clock.add_sem_waits(
            carrier, ScopedClock({None: tick_clock.global_clock}))
        si = carrier.sync_info
        if si is not None and si.on_wait and len(si.on_wait) > 1:
            waits = list(si.on_wait)
            carrier.sync_info = mybir.SyncInfo(
                on_wait=[waits[0]], on_update=list(si.on_update or []))
            for w in waits[1:]:
                n2 = self.nc.sync.nop().ins
                n2.sync_info = mybir.SyncInfo(on_wait=[w], on_update=[])
        self.nc.sync.drain()
        self.nc.all_engine_barrier()
        assert self.sems is not None
        popped = self.nc._tile_sem_poison_stack.pop()
        assert popped is self._sem_poison
        self.nc.clear_and_free_semaphores(list(self.sems.allocated().values()))
        self.nc.all_engine_barrier()


BF16 = mybir.dt.bfloat16
FP32 = mybir.dt.float32
INT32 = mybir.dt.int32
AF = mybir.ActivationFunctionType
OP = mybir.AluOpType
AX = mybir.AxisListType

B, S, L = 32, 64, 128
TOTAL = 1024
V, E, F, W, H = 30000, 300, 256, 5, 256
T = L - W + 1            # 124 valid conv positions
NCORES = 8
DPC = B // NCORES        # 4 docs per core
ECH = [128, 128, 44]     # E=300 split into K-chunks
bf16 = ml_dtypes.bfloat16


def build_nc(S_eff: int, n_groups: int, group_targets, debug_taps=False):
    """group_targets[g] = list of 4 entries: (t, d) slot or None (pad),
    shared by all cores."""
    NLOC = n_groups * 4
    S4 = S_eff * DPC
    n_blocks = (S_eff + 3) // 4
    blocks = [(4 * p, min(4 * p + 4, S_eff)) for p in range(n_blocks)]
    # group after which all of block p's sentence slots have been scattered
    ready_after = []
    for (t_lo, t_hi) in blocks:
        g_max = -1
        for g in range(n_groups):
            for td in group_targets[g]:
                if td is not None and t_lo <= td[0] < t_hi:
                    g_max = max(g_max, g)
        ready_after.append(g_max)

    nc = bass.Bass()
    # ---- DRAM I/O ----
    d_idx = nc.dram_tensor("idx_t", [128, NLOC], INT32, kind="ExternalInput")
    d_wemb = nc.dram_tensor("wemb", [V, E], BF16, kind="ExternalInput")
    d_wconv = nc.dram_tensor("wconvT", [128, W, 3, 2, 128], BF16, kind="ExternalInput")
    d_bconv = nc.dram_tensor("bconv_t", [128, 2], FP32, kind="ExternalInput")
    d_wa0 = nc.dram_tensor("wa0_t", [128, 2, 2, 128], BF16, kind="ExternalInput")
    d_ba0 = nc.dram_tensor("ba0_t", [128, 2], FP32, kind="ExternalInput")
    d_wih = nc.dram_tensor("wih_t", [128, 2, 8, 128], BF16, kind="ExternalInput")
    d_bihhh = nc.dram_tensor("bihhh_t", [128, 8], FP32, kind="ExternalInput")
    d_whh = nc.dram_tensor("whh_t", [128, 2, 8, 128], BF16, kind="ExternalInput")
    d_wa1 = nc.dram_tensor("wa1_t", [128, 2, 2, 128], BF16, kind="ExternalInput")
    d_ba1 = nc.dram_tensor("ba1_t", [128, 2], FP32, kind="ExternalInput")
    d_mb1 = nc.dram_tensor("mb1", [1, S4], FP32, kind="ExternalInput")
    d_wo = nc.dram_tensor("wo_t", [128, 2], BF16, kind="ExternalInput")
    d_boh = nc.dram_tensor("bo_half", [1, 1], FP32, kind="ExternalInput")
    d_out = nc.dram_tensor("out", [1, DPC], FP32, kind="ExternalOutput")

    with _TC(nc) as tc, ExitStack() as ctx:
        consts = ctx.enter_context(tc.tile_pool(name="consts", bufs=1))

        ident = consts.tile([128, 128], BF16)
        make_identity(nc, ident[:, :])
        idx_sb = consts.tile([128, NLOC], INT32)
        nc.sync.dma_start(out=idx_sb[:, :], in_=d_idx[:, :])
        wconv_sb = consts.tile([128, W, 3, 2, 128], BF16)
        nc.sync.dma_start(out=wconv_sb[:, :, :, :, :], in_=d_wconv[:, :, :, :, :])
        bconv_sb = consts.tile([128, 2], FP32)
        nc.sync.dma_start(out=bconv_sb[:, :], in_=d_bconv[:, :])
        wa0_sb = consts.tile([128, 2, 2, 128], BF16)
        nc.sync.dma_start(out=wa0_sb[:, :, :, :], in_=d_wa0[:, :, :, :])
        ba0_sb = consts.tile([128, 2], FP32)
        nc.sync.dma_start(out=ba0_sb[:, :], in_=d_ba0[:, :])
        wih_sb = consts.tile([128, 2, 8, 128], BF16)
        nc.sync.dma_start(out=wih_sb[:, :, :, :], in_=d_wih[:, :, :, :])
        bihhh_sb = consts.tile([128, 8], FP32)
        nc.sync.dma_start(out=bihhh_sb[:, :], in_=d_bihhh[:, :])
        whh_sb = consts.tile([128, 2, 8, 128], BF16)
        nc.sync.dma_start(out=whh_sb[:, :, :, :], in_=d_whh[:, :, :, :])
        wa1_sb = consts.tile([128, 2, 2, 128], BF16)
        nc.sync.dma_start(out=wa1_sb[:, :, :, :], in_=d_wa1[:, :, :, :])
        ba1_sb = consts.tile([128, 2], FP32)
        nc.sync.dma_start(out=ba1_sb[:, :], in_=d_ba1[:, :])
        mb1_sb = consts.tile([128, S4], FP32)
        nc.sync.dma_start(
            out=mb1_sb[:, :], in_=d_mb1[:, :].to_broadcast([128, S4]))
        wo_sb = consts.tile([128, 2], BF16)
        nc.sync.dma_start(out=wo_sb[:, :], in_=d_wo[:, :])
        boh_sb = consts.tile([1, 1], FP32)
        nc.sync.dma_start(out=boh_sb[:, :], in_=d_boh[:, :])

        # persistent state
        s0T_sb = consts.tile([128, 2, S_eff, DPC], BF16)
        nc.vector.memset(s0T_sb[:, :, :, :], 0.0)
        s0scr = consts.tile([128, 2, 4], BF16)
        hsb_sb = consts.tile([128, S_eff, 2, DPC], BF16)
        h_init = consts.tile([128, 2, DPC], BF16)
        nc.vector.memset(h_init[:, :, :], 0.0)
        c_sb = consts.tile([128, 8], FP32)
        nc.vector.memset(c_sb[:, :], 0.0)

        # ============ Phase A groups with B (x-proj) + C (LSTM) woven in ====
        with (
            nc.named_scope("phaseABC"),
            tc.tile_pool(name="emb", bufs=3) as p_emb,
            tc.tile_pool(name="ebts", bufs=2) as p_ebts,
            tc.tile_pool(name="acts", bufs=2) as p_acts,
            tc.tile_pool(name="small", bufs=2) as p_small,
            tc.tile_pool(name="xps", bufs=2) as p_xps,
            tc.tile_pool(name="lst", bufs=2) as p_lst,
            tc.tile_pool(name="ebtp", bufs=1, space="PSUM") as p_ebtp,
            tc.tile_pool(name="mmp", bufs=4, space="PSUM") as p_mmp,
            tc.tile_pool(name="xpp", bufs=1, space="PSUM") as p_xpp,
            tc.tile_pool(name="gp", bufs=1, space="PSUM") as p_gp,
        ):
            xpt_blks = {}

            def emit_group(g):
                # ---- gather 4 sentences of embeddings: [tok, sent, E] bf16
                emb_g = p_emb.tile([128, 4, E], BF16, tag="emb")
                for s in range(4):
                    nc.gpsimd.indirect_dma_start(
                        out=emb_g[:, s, :],
                        out_offset=None,
                        in_=d_wemb[:, :],
                        in_offset=IndirectOffsetOnAxis(
                            ap=idx_sb[:, 4 * g + s:4 * g + s + 1], axis=0),
                    )
                # ---- PE transpose to ebT [E-chunk, 4*128 tok], bf16 PSUM
                eb01_ps = p_ebtp.tile([128, 2, 512], BF16, tag="eb01")
                eb2_ps = p_ebtp.tile([128, 2, 512], BF16, tag="eb2")
                for s in range(4):
                    for ec in range(2):
                        nc.tensor.transpose(
                            out=eb01_ps[:, ec, 128 * s:128 * (s + 1)],
                            in_=emb_g[:, s, 128 * ec:128 * (ec + 1)],
                            identity=ident[:, :])
                    nc.tensor.transpose(
                        out=eb2_ps[:44, 0, 128 * s:128 * (s + 1)],
                        in_=emb_g[:, s, 256:300],
                        identity=ident[:, :])
                ebt_sb = p_ebts.tile([128, 3, 512], BF16, tag="ebts")
                nc.vector.tensor_copy(
                    out=ebt_sb[:, 0:2, :], in_=eb01_ps[:, :, :])
                nc.scalar.activation(
                    out=ebt_sb[:44, 2, :], in_=eb2_ps[:44, 0, :],
                    func=AF.Identity)
                # ---- conv: accumulate 5 shifted matmuls x 3 K-chunks ----
                c0_sb = p_acts.tile([128, 2, 512], BF16, tag="c0")
                for fc in range(2):
                    c0_ps = p_mmp.tile([128, 512], FP32, tag="mm")
                    ki = 0
                    for w in range(W):
                        for ec in range(3):
                            ecw = ECH[ec]
                            nc.tensor.matmul(
                                out=c0_ps[:, 0:508],
                                lhsT=wconv_sb[:ecw, w, ec, fc, :],
                                rhs=ebt_sb[:ecw, ec, w:w + 508],
                                start=(ki == 0), stop=(ki == 14),
                            )
                            ki += 1
                    nc.scalar.activation(
                        out=c0_sb[:, fc, 0:508], in_=c0_ps[:, 0:508],
                        func=AF.Tanh, bias=bconv_sb[:, fc:fc + 1])
                # ---- attn0 logits: Wa0.T @ c0T, tanh ----
                lg_sb = p_acts.tile([128, 2, 512], BF16, tag="lg")
                for mc in range(2):
                    lg_ps = p_mmp.tile([128, 512], FP32, tag="mm")
                    for kc in range(2):
                        nc.tensor.matmul(
                            out=lg_ps[:, 0:508],
                            lhsT=wa0_sb[:, kc, mc, :],
                            rhs=c0_sb[:, kc, 0:508],
                            start=(kc == 0), stop=(kc == 1),
                        )
                    nc.scalar.activation(
                        out=lg_sb[:, mc, 0:508], in_=lg_ps[:, 0:508],
                        func=AF.Tanh, bias=ba0_sb[:, mc:mc + 1])
                # ---- batched softmax over tokens + weighted sum ----
                ex_sb = p_acts.tile([128, 2, 512], BF16, tag="ex")
                den_sb = p_small.tile([128, 2, 4], FP32, tag="den")
                num_sb = p_small.tile([128, 2, 4], FP32, tag="num")
                for mc in range(2):
                    nc.scalar.activation(
                        out=ex_sb[:, mc, 0:508], in_=lg_sb[:, mc, 0:508],
                        func=AF.Exp)
                    exv = ex_sb[:, mc, :].rearrange("p (s t) -> p s t", s=4)
                    nc.vector.tensor_reduce(
                        out=den_sb[:, mc, :], in_=exv[:, :, 0:T],
                        axis=AX.X, op=OP.add)
                    prod_sb = p_small.tile([128, 512], BF16, tag="prod")
                    nc.vector.tensor_tensor(
                        out=prod_sb[:, 0:508],
                        in0=ex_sb[:, mc, 0:508], in1=c0_sb[:, mc, 0:508],
                        op=OP.mult)
                    pv = prod_sb.rearrange("p (s t) -> p s t", s=4)
                    nc.vector.tensor_reduce(
                        out=num_sb[:, mc, :], in_=pv[:, :, 0:T],
                        axis=AX.X, op=OP.add)
                nc.vector.reciprocal(
                    out=den_sb[:, :, :], in_=den_sb[:, :, :])
                # ---- s0 = num/den scattered to (t, d) slots ----
                tgt = group_targets[g]
                runs = _target_runs(tgt)
                for mc in range(2):
                    for (s_lo, n_run, td) in runs:
                        if td is None:
                            out_ap = s0scr[:, mc, s_lo:s_lo + n_run]
                        else:
                            t0, d0 = td
                            out_ap = s0T_sb[:, mc, t0:t0 + n_run, d0:d0 + 1]
                        nc.vector.tensor_tensor(
                            out=out_ap,
                            in0=num_sb[:, mc, s_lo:s_lo + n_run],
                            in1=den_sb[:, mc, s_lo:s_lo + n_run],
                            op=OP.mult)

            def emit_xproj(p):
                t_lo, t_hi = blocks[p]
                cols = (t_hi - t_lo) * DPC
                xpt_blk = p_xps.tile([128, 8, 16], FP32, tag="xpt")
                xp_ps = p_xpp.tile([128, 8, 16], FP32, tag="xp")
                for gt in range(8):
                    for kc in range(2):
                        nc.tensor.matmul(
                            out=xp_ps[:, gt, 0:cols],
                            lhsT=wih_sb[:, kc, gt, :],
                            rhs=s0T_sb[:, kc, t_lo:t_hi, :],
                            start=(kc == 0), stop=(kc == 1),
                        )
                    nc.scalar.activation(
                        out=xpt_blk[:, gt, 0:cols], in_=xp_ps[:, gt, 0:cols],
                        func=AF.Identity, bias=bihhh_sb[:, gt:gt + 1])
                xpt_blks[p] = xpt_blk

            def emit_step(p, t):
                t_lo, _ = blocks[p]
                j = t - t_lo
                xpt_blk = xpt_blks[p]
                hprev = h_init if t == 0 else hsb_sb[:, t - 1, :, :]
                g_ps = p_gp.tile([128, 32], FP32, tag="g")
                for gt in range(8):
                    for kc in range(2):
                        nc.tensor.matmul(
                            out=g_ps[:, 4 * gt:4 * gt + 4],
                            lhsT=whh_sb[:, kc, gt, :],
                            rhs=hprev[:, kc, :],
                            start=(kc == 0), stop=(kc == 1),
                        )
                ga = p_lst.tile([128, 32], FP32, tag="ga")
                nc.vector.tensor_tensor(
                    out=ga[:, :], in0=g_ps[:, :],
                    in1=xpt_blk[:, :, 4 * j:4 * (j + 1)], op=OP.add)
                # gate order (host-permuted): i f o g
                nc.scalar.activation(out=ga[:, 0:24], in_=ga[:, 0:24],
                                     func=AF.Sigmoid)
                nc.scalar.activation(out=ga[:, 24:32], in_=ga[:, 24:32],
                                     func=AF.Tanh)
                tmp = p_lst.tile([128, 8], FP32, tag="tmp")
                nc.vector.tensor_tensor(
                    out=tmp[:, :], in0=ga[:, 0:8], in1=ga[:, 24:32],
                    op=OP.mult)
                nc.vector.tensor_tensor(
                    out=c_sb[:, :], in0=c_sb[:, :], in1=ga[:, 8:16],
                    op=OP.mult)
                nc.vector.tensor_tensor(
                    out=c_sb[:, :], in0=c_sb[:, :], in1=tmp[:, :], op=OP.add)
                tch = p_lst.tile([128, 8], FP32, tag="tch")
                nc.scalar.activation(out=tch[:, :], in_=c_sb[:, :],
                                     func=AF.Tanh)
                nc.vector.tensor_tensor(
                    out=hsb_sb[:, t, :, :], in0=ga[:, 16:24], in1=tch[:, :],
                    op=OP.mult)

            # unit schedule: B_p after group ready_after[p]+1, steps spread
            # one per following group; remainder trails after the loop.
            uq = []
            for p in range(n_blocks):
                uq.append((ready_after[p] + 1, ("B", p)))
                for j, t in enumerate(range(blocks[p][0], blocks[p][1])):
                    uq.append((ready_after[p] + 1 + j, ("C", p, t)))
            ui = 0
            for g in range(n_groups):
                emit_group(g)
                while ui < len(uq) and uq[ui][0] <= g:
                    u = uq[ui][1]
                    if u[0] == "B":
                        emit_xproj(u[1])
                    else:
                        emit_step(u[1], u[2])
                    ui += 1
            while ui < len(uq):
                u = uq[ui][1]
                if u[0] == "B":
                    emit_xproj(u[1])
                else:
                    emit_step(u[1], u[2])
                ui += 1

        # ================= Phase D: sentence attention + output ============
        with (
            nc.named_scope("phaseD"),
            tc.tile_pool(name="a1p", bufs=2, space="PSUM") as p_a1p,
            tc.tile_pool(name="a1s", bufs=2) as p_a1s,
        ):
            l1_sb = p_a1s.tile([128, 2, S4], FP32, tag="l1")
            for mc in range(2):
                l1_ps = p_a1p.tile([128, S4], FP32, tag="l1p")
                for kc in range(2):
                    nc.tensor.matmul(
                        out=l1_ps[:, :],
                        lhsT=wa1_sb[:, kc, mc, :],
                        rhs=hsb_sb[:, :, kc, :],
                        start=(kc == 0), stop=(kc == 1),
                    )
                nc.scalar.activation(
                    out=l1_sb[:, mc, :], in_=l1_ps[:, :],
                    func=AF.Tanh, bias=ba1_sb[:, mc:mc + 1])
                nc.vector.tensor_tensor(
                    out=l1_sb[:, mc, :], in0=l1_sb[:, mc, :],
                    in1=mb1_sb[:, :], op=OP.add)
            ex1 = p_a1s.tile([128, 2, S_eff, DPC], FP32, tag="ex1")
            den1 = p_a1s.tile([128, 2, DPC], FP32, tag="den1")
            num1 = p_a1s.tile([128, 2, DPC], FP32, tag="num1")
            prod1 = p_a1s.tile([128, S_eff, DPC], FP32, tag="prod1")
            l1v = l1_sb.rearrange("p m (t d) -> p m t d", d=DPC)
            for mc in range(2):
                nc.scalar.activation(
                    out=ex1[:, mc, :, :], in_=l1v[:, mc, :, :], func=AF.Exp)
                nc.vector.tensor_reduce(
                    out=den1[:, mc, :],
                    in_=ex1[:, mc].rearrange("p t d -> p d t"),
                    axis=AX.X, op=OP.add)
                nc.vector.tensor_tensor(
                    out=prod1[:, :, :], in0=ex1[:, mc, :, :],
                    in1=hsb_sb[:, :, mc, :], op=OP.mult)
                nc.vector.tensor_reduce(
                    out=num1[:, mc, :],
                    in_=prod1.rearrange("p t d -> p d t"),
                    axis=AX.X, op=OP.add)
            nc.vector.reciprocal(out=den1[:, :, :], in_=den1[:, :, :])
            s1_sb = p_a1s.tile([128, 2, DPC], BF16, tag="s1")
            nc.vector.tensor_tensor(
                out=s1_sb[:, :, :], in0=num1[:, :, :], in1=den1[:, :, :],
                op=OP.mult)
            o_ps = p_a1p.tile([128, DPC], FP32, tag="op")
            for kc in range(2):
                nc.tensor.matmul(
                    out=o_ps[:1, :],
                    lhsT=wo_sb[:, kc:kc + 1],
                    rhs=s1_sb[:, kc, :],
                    start=(kc == 0), stop=(kc == 1),
                )
            y_sb = p_a1s.tile([1, DPC], FP32, tag="y")
            nc.scalar.activation(
                out=y_sb[:, :], in_=o_ps[:1, :],
                func=AF.Tanh, bias=boh_sb[:1, :1], scale=0.5)
            nc.vector.tensor_scalar(
                out=y_sb[:, :], in0=y_sb[:, :],
                scalar1=0.5, scalar2=0.5, op0=OP.mult, op1=OP.add)
            nc.sync.dma_start(out=d_out[:, :], in_=y_sb[:, :])

    return nc


def _target_runs(tgt):
    """Compress 4 per-sentence (t, d)/None targets into (start, len, td) runs
    where a run covers consecutive t at fixed d (or None-pads)."""
    runs = []
    i = 0
    while i < 4:
        if tgt[i] is None:
            j = i
            while j < 4 and tgt[j] is None:
                j += 1
            runs.append((i, j - i, None))
            i = j
        else:
            t0, d0 = tgt[i]
            j = i + 1
            while j < 4 and tgt[j] is not None and tgt[j] == (t0 + (j - i), d0):
                j += 1
            runs.append((i, j - i, (t0, d0)))
            i = j
    return runs


# reference gate order (i, f, g, o) -> kernel gate order (i, f, o, g)
_GATE_PERM = np.concatenate([
    np.arange(0, 2 * H), np.arange(3 * H, 4 * H), np.arange(2 * H, 3 * H)])


def _host_prep(inputs):
    inp = {k: np.asarray(v) for k, v in inputs.items()}
    tok = inp["input"].astype(np.int32)
    num_sent = inp["num_sent"].astype(np.int64)
    mask = np.asarray(inp["mask"], np.float32)

    S_eff = max(int(num_sent.max()), 1)
    # ragged mapping exactly like the reference scatter
    batch_ids = np.repeat(np.arange(B), num_sent)
    if len(batch_ids) < TOTAL:
        batch_ids = np.concatenate(
            [batch_ids, np.full(TOTAL - len(batch_ids), B - 1, np.int64)])
    batch_ids = batch_ids[:TOTAL]
    offsets = np.cumsum(num_sent) - num_sent
    pos = np.arange(TOTAL) - offsets[batch_ids]
    valid = pos < num_sent[batch_ids]

    per_core = []
    for k in range(NCORES):
        sids = np.where((batch_ids // DPC == k) & valid)[0]
        ents = [(int(j), int(batch_ids[j] % DPC), int(pos[j])) for j in sids]
        # position-major: all docs' sentences for a 4-slot block together,
        # so the LSTM over that block can start as soon as the block's
        # conv groups are done.
        ents.sort(key=lambda e: (e[2] // 4, e[1], e[2] % 4))
        per_core.append(ents)
    n_groups = max(1, (max(len(pc) for pc in per_core) + 3) // 4)
    NLOC = n_groups * 4

    # per-core group target maps; must agree across cores for the shared
    # program (true for uniform num_sent). Fall back handled by caller.
    tmaps = []
    for k in range(NCORES):
        tm = []
        for j in range(NLOC):
            if j < len(per_core[k]):
                _, d, p = per_core[k][j]
                tm.append((p, d))
            else:
                tm.append(None)
        tmaps.append(tm)
    uniform = all(tm == tmaps[0] for tm in tmaps)

    group_targets = [tmaps[0][4 * g:4 * g + 4] for g in range(n_groups)]

    in_maps = []
    wemb = np.asarray(inp["Wemb"], np.float32).astype(bf16)
    wconvT = np.zeros((128, W, 3, 2, 128), bf16)
    wc = np.asarray(inp["Wconv"], np.float32)  # [F,1,W,E]
    for ec in range(3):
        ecw = ECH[ec]
        for fc in range(2):
            # [p, w, m] = Wconv[fc*128+m, 0, w, ec*128+p]
            blk = wc[128 * fc:128 * (fc + 1), 0, :, 128 * ec:128 * ec + ecw]
            wconvT[:ecw, :, ec, fc, :] = blk.transpose(2, 1, 0).astype(bf16)
    bconv_t = np.asarray(inp["bconv"], np.float32).reshape(2, 128).T.copy()
    wa0_t = _pack_kx(inp["Wa0"])
    ba0_t = np.asarray(inp["ba0"], np.float32).reshape(2, 128).T.copy()
    wih_t = _pack_kx(np.asarray(inp["Wih"], np.float32).T[:, _GATE_PERM])
    whh_t = _pack_kx(np.asarray(inp["Whh"], np.float32).T[:, _GATE_PERM])
    bihhh_t = (np.asarray(inp["bih"], np.float32)
               + np.asarray(inp["bhh"], np.float32))[_GATE_PERM]
    bihhh_t = bihhh_t.reshape(8, 128).T.copy()
    wa1_t = _pack_kx(inp["Wa1"])
    ba1_t = np.asarray(inp["ba1"], np.float32).reshape(2, 128).T.copy()
    wo_t = np.asarray(inp["Wo"], np.float32).reshape(2, 128).T.astype(bf16).copy()
    bo_half = (0.5 * np.asarray(inp["bo"], np.float32)).reshape(1, 1)

    for k in range(NCORES):
        idx_t = np.zeros((128, NLOC), np.int32)
        for j, (sj, _, _) in enumerate(per_core[k]):
            idx_t[:, j] = tok[sj]
        mb1 = np.zeros((1, S_eff * DPC), np.float32)
        for d in range(DPC):
            doc = k * DPC + d
            mvals = mask[doc, :S_eff, 0]
            mb1[0, np.arange(S_eff) * DPC + d] = np.where(mvals > 0, 0.0, -1e9)
        in_maps.append({
            "idx_t": idx_t, "wemb": wemb, "wconvT": wconvT,
            "bconv_t": bconv_t, "wa0_t": wa0_t, "ba0_t": ba0_t,
            "wih_t": wih_t, "bihhh_t": bihhh_t, "whh_t": whh_t,
            "wa1_t": wa1_t, "ba1_t": ba1_t, "mb1": mb1,
            "wo_t": wo_t, "bo_half": bo_half,
        })
    return S_eff, n_groups, group_targets, tmaps, uniform, in_maps


def _pack_kx(w):
    """[K=256, M_total] -> [128, kc, mt, 128] tile pack (bf16)."""
    w = np.asarray(w, np.float32)
    K, M = w.shape
    assert K == 256 and M % 128 == 0
    mt = M // 128
    out = np.zeros((128, 2, mt, 128), bf16)
    for kc in range(2):
        for m in range(mt):
            out[:, kc, m, :] = w[128 * kc:128 * (kc + 1),
                                 128 * m:128 * (m + 1)].astype(bf16)
    return out


_NC_CACHE = {}


def kernel(**inputs) -> np.ndarray:
    S_eff, n_groups, group_targets, tmaps, uniform, in_maps = _host_prep(inputs)

    out = np.zeros((B, 1), np.float32)
    if uniform:
        key = (S_eff, n_groups, tuple(tuple(t) if t else None
                                      for g in group_targets for t in g))
        if key not in _NC_CACHE:
            _NC_CACHE[key] = build_nc(S_eff, n_groups, group_targets)
        nc = _NC_CACHE[key]
        res = run_bass_kernel_spmd(nc, in_maps, core_ids=list(range(NCORES)))
        for k in range(NCORES):
            out[k * DPC:(k + 1) * DPC, 0] = res.results[k]["out"][0]
    else:
        # ragged fallback: per-core programs
        from concourse.bass_utils import run_bass_kernel
        for k in range(NCORES):
            gt_k = [tmaps[k][4 * g:4 * g + 4] for g in range(n_groups)]
            nc = build_nc(S_eff, n_groups, gt_k)
            r = run_bass_kernel(nc, in_maps[k], core_id=0)
            out[k * DPC:(k + 1) * DPC, 0] = r["out"][0]
    return out
